# revision 1
# baseline (speedup 1.0000x reference)
"""Trainium2 Bass kernel for nn_LASCC (sparse patch-correlation attention + top-k).

Math (per batch element b):
  x_hat = L2-normalize(x, dim=channels)
  z_p[c, n] = x_hat at the two in-patch diagonal pixels (p=0: (0,0), p=1: (1,1))
  C_p = z_p^T z_p                  (1024x1024 normalized correlation, symmetric)
  C_2 = (C_0 + C_1)/2              (avg map)
  s_q = alpha * mask * C_q
  A_q = exp(2 a_q t_q) * u[n] * u[m],  t_q = mask*C_q-ish slab, u = 1/rowsum(exp(a_q t_q))
  out pixel with patch n, map q: top-3 over m of A_q[n, m]

Slabs store t_q: t_0 = mask*C_0, t_1 = mask*C_1, t_2 = t_0 + t_1 (so
q=2 needs NO matmuls and no mask pass: a_2 = alpha/2 instead of alpha).

Log-domain top-k: order over m of A[n, m] == order of T[n, m] = t[n, m]
+ ln(u_m)/a2_q (a2_q = 2 a_q), so the F-phase is ONE fp16 2x tensor-add
+ max8; the top-3 VALUES are recovered with a tiny exp on [128, 8, 3]:
out = exp(a2_q * T_top3) * u_n.  One full-size exp per chunk remains
(row sums).  ACT functions (Exp, Ln, Square, Copy) mostly share the
natural_log_exp_and_others table.

E(k+1) and F(k) are interleaved per chunk at emission so each engine's
in-order stream alternates ready work instead of head-of-line blocking.
"""
import numpy as np

import concourse.bass as bass
import concourse.mybir as mybir
from concourse import bacc
from concourse.tile import TileContext
from concourse.bass_utils import run_bass_kernel_spmd

F32 = mybir.dt.float32
FP16 = mybir.dt.float16
AF = mybir.ActivationFunctionType
ALU = mybir.AluOpType

B_FULL = 16
N_CORES = 8
B_LOC = B_FULL // N_CORES  # 2
C = 128
H = W = 64
NPH = 32
NP = 1024
PS = 2
TOPK = 3
NCHUNK = NP // 128  # 8

N_DVE_MASK = 5   # q0/q1 chunks whose mask-mult runs on DVE (rest ACT+Pool)
N_DVE_TADD = 6   # F-phase T-adds on DVE (rest Pool)
N_DVE_S2 = 4     # q2 slab-adds on DVE (rest Pool)

LAST_EXEC_NS = None


def _free_bcast_ap(tile_ap, free_dims):
    ap = tile_ap
    new = [ap.ap[0]] + [list(d) for d in free_dims]
    return bass.AP(ap.tensor, ap.offset, new)


def build_nc():
    import concourse.bacc as _bacc_mod
    _orig_tables = _bacc_mod.get_activation_tables

    def _one_table(arch):
        t = _orig_tables(arch)
        # keep dict order (act_func_set_id = index) but leave only the
        # ln+exp superset populated so every activation shares one table
        return {k: (v if k == "natural_log_exp_and_others" else set())
                for k, v in t.items()}

    _bacc_mod.get_activation_tables = _one_table
    try:
        return _build_nc_inner()
    finally:
        _bacc_mod.get_activation_tables = _orig_tables


def _build_nc_inner():
    nc = bacc.Bacc(trn_type="TRN2")

    x_d = nc.dram_tensor("x", [B_LOC, C, H * W], F32, kind="ExternalInput")
    alpha_d = nc.dram_tensor("alpha", [128, 1], F32, kind="ExternalInput")
    mask_d = nc.dram_tensor("mask", [NP, NP], FP16, kind="ExternalInput")
    out_d = nc.dram_tensor("out", [B_LOC, 3, NP, TOPK], F32, kind="ExternalOutput")

    with TileContext(nc) as tc:
        with tc.tile_pool(name="const", bufs=1) as cpool, \
             tc.tile_pool(name="zp", bufs=1) as zpool, \
             tc.tile_pool(name="slab", bufs=2) as slabp, \
             tc.tile_pool(name="ssl", bufs=5) as sslp, \
             tc.tile_pool(name="work", bufs=3) as work, \
             tc.tile_pool(name="wsc", bufs=3) as wscp, \
             tc.tile_pool(name="small", bufs=3) as small, \
             tc.tile_pool(name="ps", bufs=2, space="PSUM") as ps, \
             tc.tile_pool(name="psn", bufs=2, space="PSUM") as psn, \
             tc.tile_pool(name="dsc", bufs=3, space="DRAM") as dsc:

            # ---- constants
            ones_k = cpool.tile([128, 1], FP16)   # colsum matmul lhsT
            nc.vector.memset(ones_k, 1.0)
            ones_r = cpool.tile([1, 128], FP16)   # K=1 bcast matmul lhsT
            nc.vector.memset(ones_r, 1.0)
            av = cpool.tile([128, 1], F32)        # alpha
            nc.sync.dma_start(av, alpha_d[:, :])
            av_h = cpool.tile([128, 1], F32)      # alpha/2
            nc.vector.tensor_scalar_mul(av_h, av, 0.5)
            av_d = cpool.tile([128, 1], F32)      # 2*alpha
            nc.vector.tensor_scalar_mul(av_d, av, 2.0)
            rav2 = cpool.tile([128, 1], F32)      # 1/(2*alpha)
            nc.vector.reciprocal(rav2, av_d)
            rav1 = cpool.tile([128, 1], F32)      # 1/alpha
            nc.vector.reciprocal(rav1, av)
            scale_E = [av, av, av_h]      # a_q for the rowsum exp
            scale_T = [av_d, av_d, av]    # 2 a_q for the tiny value exp
            scale_L = [rav2, rav2, rav1]  # 1/(2 a_q) for ln(u)

            # ---- mask (fp16, [p, i, m] chunk layout)
            mask_sb = cpool.tile([128, NCHUNK, NP], FP16)
            nc.sync.dma_start(
                mask_sb, mask_d[:, :].rearrange("(i p) m -> p i m", p=128))

            # ---- phase N
            chains = []
            for b in range(B_LOC):
                xs = slabp.tile([128, H * W], F32, name=f"xs{b}", tag="xs")
                nc.sync.dma_start(xs, x_d[b])
                xr = xs.rearrange("c (i r j s) -> c r s i j", r=PS, s=PS, j=NPH)
                for p in range(PS):
                    chains.append((b, p, xr[:, p, p]))

            inv = {}
            nrms = {}
            for b, p, zv in chains:  # nrm2 via DVE square + PE colsum
                zsq = work.tile([128, NP], FP16, name="zsq", tag="zsq", bufs=2)
                nc.vector.tensor_tensor(
                    out=zsq.rearrange("c (a b) -> c a b", a=NPH),
                    in0=zv, in1=zv, op=ALU.mult)
                nrm = psn.tile([1, NP], F32, name="nrm", tag="nrm", bufs=1)
                for h in range(2):
                    nc.tensor.matmul(nrm[:, 512 * h:512 * (h + 1)], ones_k,
                                     zsq[:, 512 * h:512 * (h + 1)],
                                     start=True, stop=True)
                nrms[(b, p)] = nrm
            lnns = {}
            for b, p, zv in chains:  # cluster the Lns, then the Exps
                lnn = small.tile([1, NP], F32, name="lnn", tag="lnn", bufs=2)
                nc.scalar.activation(lnn, nrms[(b, p)], AF.Ln)
                lnns[(b, p)] = lnn
            for b, p, zv in chains:  # inv = exp(-0.5 ln nrm2)
                inv1 = small.tile([1, NP], FP16, name="inv1", tag="inv1",
                                  bufs=2)
                nc.scalar.activation(inv1, lnns[(b, p)], AF.Exp, scale=-0.5)
                inv[(b, p)] = inv1
            zp = {}
            for b, p, zv in chains:
                ibc = psn.tile([128, NP], F32, name="ibc", tag="ibc", bufs=1)
                for h in range(2):
                    nc.tensor.matmul(ibc[:, 512 * h:512 * (h + 1)], ones_r,
                                     inv[(b, p)][:, 512 * h:512 * (h + 1)],
                                     start=True, stop=True)
                z = zpool.tile([128, NP], FP16, name=f"z{b}{p}", tag=f"z{b}{p}",
                               bufs=1)
                nc.vector.tensor_tensor(
                    out=z.rearrange("c (a b) -> c a b", a=NPH), in0=zv,
                    in1=ibc.rearrange("c (a b) -> c a b", a=NPH), op=ALU.mult)
                zp[(b, p)] = z

            # ---- phase M: 6 stages, per-chunk interleaved software pipeline
            s_of = {}  # (b, q) -> slab

            def E_chunk(b, q, i, s_sl, rT):
                if q < 2:
                    zs = zp[(b, q)]
                    G = ps.tile([128, NP], F32, name="G", tag="G")
                    for h in range(2):
                        nc.tensor.matmul(
                            G[:, 512 * h:512 * (h + 1)],
                            zs[:, 128 * i:128 * (i + 1)],
                            zs[:, 512 * h:512 * (h + 1)],
                            start=True, stop=True)
                    # mask==1 outside a 512-wide band around the diagonal
                    # blocks: multiply only the band (DVE), copy the rest
                    c0 = min(max(128 * i - 192, 0), 512)
                    nc.vector.tensor_tensor(
                        out=s_sl[:, i, c0:c0 + 512],
                        in0=G[:, c0:c0 + 512],
                        in1=mask_sb[:, i, c0:c0 + 512], op=ALU.mult)
                    for a, bnd in ((0, c0), (c0 + 512, NP)):
                        if bnd > a:
                            nc.scalar.copy(s_sl[:, i, a:bnd], G[:, a:bnd])
                else:
                    s0, s1 = s_of[(b, 0)], s_of[(b, 1)]
                    if i % 4 == 0:  # batched 4-chunk add on DVE
                        nc.vector.tensor_tensor(
                            out=s_sl[:, i:i + 4, :], in0=s0[:, i:i + 4, :],
                            in1=s1[:, i:i + 4, :], op=ALU.add)
                e_scr = work.tile([128, NP], FP16, name="e_scr", tag="e_scr",
                                  bufs=2)
                nc.scalar.activation(e_scr, s_sl[:, i, :], AF.Exp,
                                     scale=scale_E[q],
                                     accum_out=rT[:, i:i + 1])

            def E_tail(b, q, s_sl, rT):
                u8 = small.tile([128, NCHUNK], F32, name="u8", tag="u8", bufs=4)
                nc.vector.reciprocal(u8, rT)
                lnu = small.tile([128, NCHUNK], F32, name="lnu", tag="lnu")
                nc.scalar.activation(lnu, u8, AF.Ln)
                lnsc = small.tile([128, NCHUNK], FP16, name="lnsc", tag="lnsc")
                nc.vector.tensor_scalar_mul(lnsc, lnu, scale_L[q][:, 0:1])
                l_dram = dsc.tile([NP], FP16, name="l_dram", tag="l_dram")
                nc.sync.dma_start(
                    l_dram[:].rearrange("(i p) -> p i", p=128), lnsc)
                lnrow = small.tile([1, NP], FP16, name="lnrow", tag="lnrow",
                                   bufs=2)
                nc.sync.dma_start(
                    lnrow, l_dram[:].rearrange("(a m) -> a m", a=1))
                lnps = psn.tile([128, NP], F32, name="lnps", tag="ibc",
                                bufs=1)
                for h in range(2):
                    nc.tensor.matmul(lnps[:, 512 * h:512 * (h + 1)], ones_r,
                                     lnrow[:, 512 * h:512 * (h + 1)],
                                     start=True, stop=True)
                lnubc = wscp.tile([128, NP], FP16, name="lnubc", tag="lnubc",
                                  bufs=3)
                nc.scalar.copy(lnubc, lnps)
                return dict(s_sl=s_sl, lnubc=lnubc, u8=u8, b=b, q=q)

            def F_thalf(stg, h, t8):
                # batched T-add over 4 chunks: T = s + ln(u_m)/(2 a_q)
                # (Pool engine: its software Add runs near 1 elem/cycle)
                T_bat = wscp.tile([128, 4, NP], FP16, name=f"T_bat{h}",
                                  tag=f"T_bat{h}", bufs=2)
                lnb = _free_bcast_ap(stg["lnubc"][:, :], [[0, 4], [1, NP]])
                nc.vector.tensor_tensor(
                    out=T_bat, in0=stg["s_sl"][:, 4 * h:4 * (h + 1), :],
                    in1=lnb, op=ALU.add)
                return T_bat

            def F_max8(stg, i, T_bat, t8):
                nc.vector.max(out=t8[:, i, :], in_=T_bat[:, i % 4, :])

            def F_tail(stg, t8):
                b, q = stg["b"], stg["q"]
                tex = small.tile([128, NCHUNK, TOPK], F32, name="tex",
                                 tag="tex")
                nc.scalar.activation(tex, t8[:, :, :TOPK], AF.Exp,
                                     scale=scale_T[q])
                oacc = small.tile([128, NCHUNK, TOPK], F32, name="oacc",
                                  tag="oacc")
                u8b = _free_bcast_ap(stg["u8"][:, :], [[1, NCHUNK], [0, TOPK]])
                nc.vector.tensor_tensor(out=oacc, in0=tex, in1=u8b,
                                        op=ALU.mult)
                dst = out_d[b, q].rearrange("(i p) k -> p i k", p=128)
                nc.sync.dma_start(dst, oacc)

            def emit_F_interleaved(pstg, pt8, echunk_fn):
                """Interleave F(prev) pieces between E(next) chunk emissions."""
                Tb = None
                for i in range(NCHUNK):
                    if echunk_fn is not None:
                        echunk_fn(i)
                    if i % 4 == 0:
                        Tb = F_thalf(pstg, i // 4, pt8)
                    F_max8(pstg, i, Tb, pt8)
                F_tail(pstg, pt8)

            stages = [(b, q) for b in range(B_LOC) for q in range(3)]
            fq = []   # (stg, t8) queue awaiting F emission
            for (b, q) in stages:
                s_sl = sslp.tile([128, NCHUNK, NP], FP16, name="s_sl",
                                 tag="s_sl")
                s_of[(b, q)] = s_sl
                rT = small.tile([128, NCHUNK], F32, name="rT", tag="rT")
                if len(fq) >= 2:
                    pstg, pt8 = fq.pop(0)
                    emit_F_interleaved(pstg, pt8,
                                       lambda i: E_chunk(b, q, i, s_sl, rT))
                else:
                    for i in range(NCHUNK):
                        E_chunk(b, q, i, s_sl, rT)
                stg = E_tail(b, q, s_sl, rT)
                t8 = small.tile([128, NCHUNK, 8], FP16, name="t8", tag="t8", bufs=4)
                fq.append((stg, t8))
            for pstg, pt8 in fq:
                emit_F_interleaved(pstg, pt8, None)

    nc.compile()
    return nc


_NC_CACHE = None


def _get_nc():
    global _NC_CACHE
    if _NC_CACHE is None:
        _NC_CACHE = build_nc()
    return _NC_CACHE


def _build_mask() -> np.ndarray:
    rat_s = np.float32(0.05)
    sr = np.float32(NPH) * rat_s
    ind_r = np.arange(NPH, dtype=np.float32).reshape(1, NPH, 1)
    ind_c = np.arange(NPH, dtype=np.float32).reshape(1, 1, NPH)
    cent = np.arange(NPH, dtype=np.float32)
    cent_r = np.repeat(cent, NPH).reshape(NP, 1, 1)
    cent_c = np.tile(cent, NPH).reshape(NP, 1, 1)
    g = np.exp(-((ind_r - cent_r) ** 2) / (2.0 * sr * sr)) * np.exp(
        -((ind_c - cent_c) ** 2) / (2.0 * sr * sr)
    )
    return (1.0 - g).reshape(NP, NP).astype(np.float16)


def kernel(x: np.ndarray, alpha: np.ndarray) -> np.ndarray:
    global LAST_EXEC_NS
    x = np.ascontiguousarray(np.asarray(x, dtype=np.float32))
    alpha_arr = np.full((128, 1), np.float32(np.asarray(alpha)),
                        dtype=np.float32)
    mask = _build_mask()

    nc = _get_nc()
    in_maps = []
    for core in range(N_CORES):
        xs = x[core * B_LOC:(core + 1) * B_LOC].reshape(B_LOC, C, H * W)
        in_maps.append({"x": np.ascontiguousarray(xs), "alpha": alpha_arr,
                        "mask": mask})
    res = run_bass_kernel_spmd(nc, in_maps, core_ids=list(range(N_CORES)))
    LAST_EXEC_NS = res.exec_time_ns

    out = np.empty((B_FULL, TOPK, H, W), dtype=np.float32)
    for core in range(N_CORES):
        t = res.results[core]["out"]
        for bl in range(B_LOC):
            bg = core * B_LOC + bl
            tq = t[bl].reshape(3, NPH, NPH, TOPK).transpose(0, 3, 1, 2)
            out[bg, :, 0::2, 0::2] = tq[0]
            out[bg, :, 1::2, 1::2] = tq[1]
            out[bg, :, 0::2, 1::2] = tq[2]
            out[bg, :, 1::2, 0::2] = tq[2]
    return out



# revision 2
# speedup vs baseline: 1.3540x; 1.3540x over previous
"""Trainium2 Bass kernel for nn_LASCC (sparse patch-correlation attention + top-k).

Math (per batch element b):
  x_hat = L2-normalize(x, dim=channels)
  z_p[c, n] = x_hat at the two in-patch diagonal pixels (p=0: (0,0), p=1: (1,1))
  C_p = z_p^T z_p                  (1024x1024 normalized correlation, symmetric)
  C_2 = (C_0 + C_1)/2              (avg map)
  s_q = alpha * mask * C_q
  A_q = softmax_row(s_q) * softmax_col(s_q); s symmetric => A = exp(2 a t) u_n u_m
  out pixel with patch n, map q: top-3 over m of A_q[n, m]

Slabs store t_q: t_0 = mask*C_0, t_1 = mask*C_1, t_2 = t_0 + t_1 (so
q=2 needs NO matmuls and no mask pass: a_2 = alpha/2 instead of alpha).

Log-domain top-k: order over m of A[n, m] == order of T[n, m] = t[n, m]
+ ln(u_m)/a2_q (a2_q = 2 a_q), so the F-phase is ONE fp16 2x tensor-add
+ max8; the top-3 VALUES are recovered with a tiny exp on [128, 8, 3]:
out = exp(a2_q * T_top3) * u_n.  One full-size exp per chunk remains
(row sums), with accum_out giving the row sums for free.

ln(u_m) is needed along the FREE (column) dim but is computed in row
layout [128, 8].  Since s is symmetric, col sums == row sums, and the
broadcast lnubc[p, 128*i+n] = lnsc[n, i] is exactly 8 PE matmuls with
lhsT = lnsc[:, i] stride-0-replicated along M and rhs = identity:
out[m, n] = sum_k lnsc[k, i] * I[k, n] = lnsc[n, i].  No DRAM round
trip, no transpose.  The broadcast matmuls + PSUM->SBUF copy for stage
k are emitted early in stage k+1's chunk loop so the PE queue never
head-of-line blocks on them.

E(k+1) and F(k) are interleaved per chunk at emission so each engine's
in-order stream alternates ready work instead of head-of-line blocking.
"""
import numpy as np

import concourse.bass as bass
import concourse.mybir as mybir
from concourse import bacc
from concourse.tile import TileContext
from concourse.bass_utils import run_bass_kernel_spmd

F32 = mybir.dt.float32
FP16 = mybir.dt.float16
AF = mybir.ActivationFunctionType
ALU = mybir.AluOpType

B_FULL = 16
N_CORES = 8
B_LOC = B_FULL // N_CORES  # 2
C = 128
H = W = 64
NPH = 32
NP = 1024
PS = 2
TOPK = 3
NCHUNK = NP // 128  # 8
BAND = 512

LAST_EXEC_NS = None


def _band_c0(i: int) -> int:
    return min(max(128 * i - 192, 0), NP - BAND)


def _free_bcast_ap(tile_ap, free_dims):
    ap = tile_ap
    new = [ap.ap[0]] + [list(d) for d in free_dims]
    return bass.AP(ap.tensor, ap.offset, new)


def build_nc():
    import concourse.bacc as _bacc_mod
    _orig_tables = _bacc_mod.get_activation_tables

    def _one_table(arch):
        t = _orig_tables(arch)
        # keep dict order (act_func_set_id = index) but leave only the
        # ln+exp superset populated so every activation shares one table
        return {k: (v if k == "natural_log_exp_and_others" else set())
                for k, v in t.items()}

    _bacc_mod.get_activation_tables = _one_table
    try:
        return _build_nc_inner()
    finally:
        _bacc_mod.get_activation_tables = _orig_tables


def _build_nc_inner():
    nc = bacc.Bacc(trn_type="TRN2")

    x_d = nc.dram_tensor("x", [B_LOC, C, H * W], F32, kind="ExternalInput")
    alpha_d = nc.dram_tensor("alpha", [128, 1], F32, kind="ExternalInput")
    mask_d = nc.dram_tensor("mask", [128, NCHUNK, BAND], FP16,
                            kind="ExternalInput")
    ident_d = nc.dram_tensor("ident", [128, 128], FP16, kind="ExternalInput")
    out_d = nc.dram_tensor("out", [B_LOC, 3, NP, TOPK], F32, kind="ExternalOutput")

    with TileContext(nc) as tc:
        with tc.tile_pool(name="const", bufs=1) as cpool, \
             tc.tile_pool(name="zp", bufs=1) as zpool, \
             tc.tile_pool(name="slab", bufs=2) as slabp, \
             tc.tile_pool(name="ssl", bufs=5) as sslp, \
             tc.tile_pool(name="work", bufs=3) as work, \
             tc.tile_pool(name="wsc", bufs=3) as wscp, \
             tc.tile_pool(name="small", bufs=3) as small, \
             tc.tile_pool(name="ps", bufs=2, space="PSUM") as ps, \
             tc.tile_pool(name="psn", bufs=2, space="PSUM") as psn:

            # ---- constants (xs0 DMA first: it gates the critical path)
            xs_t = {}
            for b in range(B_LOC):
                xs_t[b] = slabp.tile([128, H * W], F32, name=f"xs{b}",
                                     tag="xs")
            for h in range(2):  # chunked so phase N can start early
                nc.sync.dma_start(xs_t[0][:, 2048 * h:2048 * (h + 1)],
                                  x_d[0][:, 2048 * h:2048 * (h + 1)])

            ones_k = cpool.tile([128, 1], FP16)   # colsum matmul lhsT
            nc.vector.memset(ones_k, 1.0)
            ones_r = cpool.tile([1, 128], FP16)   # K=1 bcast matmul lhsT
            nc.vector.memset(ones_r, 1.0)
            av = cpool.tile([128, 1], F32)        # alpha
            nc.sync.dma_start(av, alpha_d[:, :])
            ident = cpool.tile([128, 128], FP16)
            nc.sync.dma_start(ident, ident_d[:, :])
            av_h = cpool.tile([128, 1], F32)      # alpha/2
            nc.vector.tensor_scalar_mul(av_h, av, 0.5)
            av_d = cpool.tile([128, 1], F32)      # 2*alpha
            nc.vector.tensor_scalar_mul(av_d, av, 2.0)
            rav2n = cpool.tile([128, 1], F32)     # -1/(2*alpha)
            nc.vector.reciprocal(rav2n, av_d)
            nc.vector.tensor_scalar_mul(rav2n, rav2n, -1.0)
            rav1n = cpool.tile([128, 1], F32)     # -1/alpha
            nc.vector.reciprocal(rav1n, av)
            nc.vector.tensor_scalar_mul(rav1n, rav1n, -1.0)
            scale_E = [av, av, av_h]        # a_q for the rowsum exp
            scale_T = [av_d, av_d, av]      # 2 a_q for the tiny value exp
            scale_Ln = [rav2n, rav2n, rav1n]  # -1/(2 a_q) for ln(u) from ln(R)

            # ---- mask band (fp16, [p, i, j] with j a 512 window per chunk)
            mask_sb = cpool.tile([128, NCHUNK, BAND], FP16)
            nc.sync.dma_start(mask_sb, mask_d[:, :, :])

            for h in range(2):
                nc.sync.dma_start(xs_t[1][:, 2048 * h:2048 * (h + 1)],
                                  x_d[1][:, 2048 * h:2048 * (h + 1)])

            # ---- phase N
            chains = []
            for b in range(B_LOC):
                xr = xs_t[b].rearrange("c (i r j s) -> c r s i j",
                                       r=PS, s=PS, j=NPH)
                for p in range(PS):
                    chains.append((b, p, xr[:, p, p]))

            nrms = {}
            for b, p, zv in chains:  # nrm2 via DVE square + PE colsum
                zsq = work.tile([128, NP], FP16, name="zsq", tag="zsq", bufs=2)
                nc.vector.tensor_tensor(
                    out=zsq.rearrange("c (a b) -> c a b", a=NPH),
                    in0=zv, in1=zv, op=ALU.mult)
                nrm = psn.tile([1, NP], F32, name="nrm", tag="nrm", bufs=1)
                for h in range(2):
                    nc.tensor.matmul(nrm[:, 512 * h:512 * (h + 1)], ones_k,
                                     zsq[:, 512 * h:512 * (h + 1)],
                                     start=True, stop=True)
                nrms[(b, p)] = nrm
            lnns = {}
            for b, p, zv in chains:  # cluster the Lns, then the Exps
                lnn = small.tile([1, NP], F32, name="lnn", tag="lnn", bufs=2)
                nc.scalar.activation(lnn, nrms[(b, p)], AF.Ln)
                lnns[(b, p)] = lnn
            inv = {}
            for b, p, zv in chains:  # inv = exp(-0.5 ln nrm2)
                inv1 = small.tile([1, NP], FP16, name="inv1", tag="inv1",
                                  bufs=2)
                nc.scalar.activation(inv1, lnns[(b, p)], AF.Exp, scale=-0.5)
                inv[(b, p)] = inv1
            zp = {}
            for b, p, zv in chains:
                ibc = psn.tile([128, NP], F32, name="ibc", tag="ibc", bufs=1)
                for h in range(2):
                    nc.tensor.matmul(ibc[:, 512 * h:512 * (h + 1)], ones_r,
                                     inv[(b, p)][:, 512 * h:512 * (h + 1)],
                                     start=True, stop=True)
                z = zpool.tile([128, NP], FP16, name=f"z{b}{p}", tag=f"z{b}{p}",
                               bufs=1)
                nc.vector.tensor_tensor(
                    out=z.rearrange("c (a b) -> c a b", a=NPH), in0=zv,
                    in1=ibc.rearrange("c (a b) -> c a b", a=NPH), op=ALU.mult)
                zp[(b, p)] = z

            # ---- phase M: per-chunk interleaved software pipeline
            s_of = {}  # (b, q) -> slab

            def E_chunk(b, q, i, s_sl, rT):
                if q < 2:
                    zs = zp[(b, q)]
                    G = ps.tile([128, NP], F32, name="G", tag="G")
                    for h in range(2):
                        nc.tensor.matmul(
                            G[:, 512 * h:512 * (h + 1)],
                            zs[:, 128 * i:128 * (i + 1)],
                            zs[:, 512 * h:512 * (h + 1)],
                            start=True, stop=True)
                    # mask==1 outside a 512-wide band around the diagonal
                    # blocks: multiply only the band (DVE), copy the rest
                    c0 = _band_c0(i)
                    nc.vector.tensor_tensor(
                        out=s_sl[:, i, c0:c0 + BAND],
                        in0=G[:, c0:c0 + BAND],
                        in1=mask_sb[:, i, :], op=ALU.mult)
                    for a, bnd in ((0, c0), (c0 + BAND, NP)):
                        if bnd > a:
                            nc.scalar.copy(s_sl[:, i, a:bnd], G[:, a:bnd])
                else:
                    s0, s1 = s_of[(b, 0)], s_of[(b, 1)]
                    if i % 4 == 0:  # batched 4-chunk add on DVE
                        nc.vector.tensor_tensor(
                            out=s_sl[:, i:i + 4, :], in0=s0[:, i:i + 4, :],
                            in1=s1[:, i:i + 4, :], op=ALU.add)
                e_scr = work.tile([128, NP], FP16, name="e_scr", tag="e_scr",
                                  bufs=2)
                nc.scalar.activation(e_scr, s_sl[:, i, :], AF.Exp,
                                     scale=scale_E[q],
                                     accum_out=rT[:, i:i + 1])

            def E_tail_sums(b, q, s_sl, rT):
                u8 = small.tile([128, NCHUNK], F32, name="u8", tag="u8", bufs=4)
                nc.vector.reciprocal(u8, rT)
                lnr = small.tile([128, NCHUNK], F32, name="lnr", tag="lnr")
                nc.scalar.activation(lnr, rT, AF.Ln)
                lnsc = small.tile([128, NCHUNK], FP16, name="lnsc", tag="lnsc",
                                  bufs=2)
                nc.vector.tensor_scalar_mul(lnsc, lnr, scale_Ln[q][:, 0:1])
                return dict(s_sl=s_sl, lnsc=lnsc, u8=u8, b=b, q=q)

            def E_tail_bcast(stg):
                # lnubc[p, 128*i + n] = lnsc[n, i] via 8 identity matmuls
                lnps = psn.tile([128, NP], F32, name="lnps", tag="ibc",
                                bufs=1)
                for i in range(NCHUNK):
                    lhsT = _free_bcast_ap(stg["lnsc"][:, i:i + 1], [[0, 128]])
                    nc.tensor.matmul(lnps[:, 128 * i:128 * (i + 1)],
                                     lhsT, ident, start=True, stop=True)
                lnubc = wscp.tile([128, NP], FP16, name="lnubc", tag="lnubc",
                                  bufs=3)
                nc.scalar.copy(lnubc, lnps)
                stg["lnubc"] = lnubc

            def F_thalf(stg, h, t8):
                # batched T-add over 4 chunks: T = s + ln(u_m)/(2 a_q)
                T_bat = wscp.tile([128, 4, NP], FP16, name=f"T_bat{h}",
                                  tag=f"T_bat{h}", bufs=2)
                lnb = _free_bcast_ap(stg["lnubc"][:, :], [[0, 4], [1, NP]])
                nc.vector.tensor_tensor(
                    out=T_bat, in0=stg["s_sl"][:, 4 * h:4 * (h + 1), :],
                    in1=lnb, op=ALU.add)
                return T_bat

            def F_max8(stg, i, T_bat, t8):
                nc.vector.max(out=t8[:, i, :], in_=T_bat[:, i % 4, :])

            def F_tail(stg, t8):
                b, q = stg["b"], stg["q"]
                tex = small.tile([128, NCHUNK, TOPK], F32, name="tex",
                                 tag="tex")
                nc.scalar.activation(tex, t8[:, :, :TOPK], AF.Exp,
                                     scale=scale_T[q])
                oacc = small.tile([128, NCHUNK, TOPK], F32, name="oacc",
                                  tag="oacc")
                u8b = _free_bcast_ap(stg["u8"][:, :], [[1, NCHUNK], [0, TOPK]])
                nc.vector.tensor_tensor(out=oacc, in0=tex, in1=u8b,
                                        op=ALU.mult)
                dst = out_d[b, q].rearrange("(i p) k -> p i k", p=128)
                nc.sync.dma_start(dst, oacc)

            def emit_F_interleaved(pstg, pt8, echunk_fn):
                """Interleave F(prev) pieces between E(next) chunk emissions."""
                Tb = None
                for i in range(NCHUNK):
                    if echunk_fn is not None:
                        echunk_fn(i)
                    if i % 4 == 0:
                        Tb = F_thalf(pstg, i // 4, pt8)
                    F_max8(pstg, i, Tb, pt8)
                F_tail(pstg, pt8)

            stages = [(b, q) for b in range(B_LOC) for q in range(3)]
            fq = []   # (stg, t8) queue awaiting F emission
            pend = []  # stages awaiting the lnubc broadcast emission
            for (b, q) in stages:
                s_sl = sslp.tile([128, NCHUNK, NP], FP16, name="s_sl",
                                 tag="s_sl")
                s_of[(b, q)] = s_sl
                rT = small.tile([128, NCHUNK], F32, name="rT", tag="rT")

                def echunk(i, b=b, q=q, s_sl=s_sl, rT=rT):
                    E_chunk(b, q, i, s_sl, rT)
                    if i == 1 and pend:
                        E_tail_bcast(pend.pop(0))

                if len(fq) >= 2:
                    pstg, pt8 = fq.pop(0)
                    emit_F_interleaved(pstg, pt8, echunk)
                else:
                    for i in range(NCHUNK):
                        echunk(i)
                stg = E_tail_sums(b, q, s_sl, rT)
                pend.append(stg)
                t8 = small.tile([128, NCHUNK, 8], FP16, name="t8", tag="t8",
                                bufs=4)
                fq.append((stg, t8))
            for stg in pend:
                E_tail_bcast(stg)
            for pstg, pt8 in fq:
                emit_F_interleaved(pstg, pt8, None)

    nc.compile()
    return nc


_NC_CACHE = None


def _get_nc():
    global _NC_CACHE
    if _NC_CACHE is None:
        _NC_CACHE = build_nc()
    return _NC_CACHE


def _build_mask_band() -> np.ndarray:
    rat_s = np.float32(0.05)
    sr = np.float32(NPH) * rat_s
    ind_r = np.arange(NPH, dtype=np.float32).reshape(1, NPH, 1)
    ind_c = np.arange(NPH, dtype=np.float32).reshape(1, 1, NPH)
    cent = np.arange(NPH, dtype=np.float32)
    cent_r = np.repeat(cent, NPH).reshape(NP, 1, 1)
    cent_c = np.tile(cent, NPH).reshape(NP, 1, 1)
    g = np.exp(-((ind_r - cent_r) ** 2) / (2.0 * sr * sr)) * np.exp(
        -((ind_c - cent_c) ** 2) / (2.0 * sr * sr)
    )
    full = (1.0 - g).reshape(NP, NP).astype(np.float16)
    band = np.empty((128, NCHUNK, BAND), dtype=np.float16)
    for i in range(NCHUNK):
        c0 = _band_c0(i)
        band[:, i, :] = full[128 * i:128 * (i + 1), c0:c0 + BAND]
    return band


def kernel(x: np.ndarray, alpha: np.ndarray) -> np.ndarray:
    global LAST_EXEC_NS
    x = np.ascontiguousarray(np.asarray(x, dtype=np.float32))
    alpha_arr = np.full((128, 1), np.float32(np.asarray(alpha)),
                        dtype=np.float32)
    mask = _build_mask_band()
    ident = np.eye(128, dtype=np.float16)

    nc = _get_nc()
    in_maps = []
    for core in range(N_CORES):
        xs = x[core * B_LOC:(core + 1) * B_LOC].reshape(B_LOC, C, H * W)
        in_maps.append({"x": np.ascontiguousarray(xs), "alpha": alpha_arr,
                        "mask": mask, "ident": ident})
    res = run_bass_kernel_spmd(nc, in_maps, core_ids=list(range(N_CORES)))
    LAST_EXEC_NS = res.exec_time_ns

    out = np.empty((B_FULL, TOPK, H, W), dtype=np.float32)
    for core in range(N_CORES):
        t = res.results[core]["out"]
        for bl in range(B_LOC):
            bg = core * B_LOC + bl
            tq = t[bl].reshape(3, NPH, NPH, TOPK).transpose(0, 3, 1, 2)
            out[bg, :, 0::2, 0::2] = tq[0]
            out[bg, :, 1::2, 1::2] = tq[1]
            out[bg, :, 0::2, 1::2] = tq[2]
            out[bg, :, 1::2, 0::2] = tq[2]
    return out


# revision 3
# speedup vs baseline: 1.5357x; 1.1342x over previous
"""Trainium2 Bass kernel for nn_LASCC (sparse patch-correlation attention + top-k).

Math (per batch element b):
  x_hat = L2-normalize(x, dim=channels)
  z_p[c, n] = x_hat at the two in-patch diagonal pixels (p=0: (0,0), p=1: (1,1))
  C_p = z_p^T z_p                  (1024x1024 normalized correlation, symmetric)
  C_2 = (C_0 + C_1)/2              (avg map)
  s_q = alpha * mask * C_q
  A_q = softmax_row(s_q) * softmax_col(s_q); s symmetric => A = exp(2 a t) u_n u_m
  out pixel with patch n, map q: top-3 over m of A_q[n, m]

Slabs store t_q: t_0 = mask*C_0, t_1 = mask*C_1, t_2 = t_0 + t_1 (so
q=2 needs NO matmuls and no mask pass: a_2 = alpha/2 instead of alpha).

Log-domain top-k: order over m of A[n, m] == order of T[n, m] = t[n, m]
+ ln(u_m)/a2_q (a2_q = 2 a_q), so the F-phase is ONE fp16 2x tensor-add
+ max8; the top-3 VALUES are recovered with a tiny exp on [128, 8, 3]:
out = exp(a2_q * T_top3) * u_n.  One full-size exp per chunk remains
(row sums), with accum_out giving the row sums for free.

ln(u_m) is needed along the FREE (column) dim but is computed in row
layout [128, 8].  Since s is symmetric, col sums == row sums, and the
broadcast lnubc[p, 128*i+n] = lnsc[n, i] is exactly 8 PE matmuls with
lhsT = lnsc[:, i] stride-0-replicated along M and rhs = identity:
out[m, n] = sum_k lnsc[k, i] * I[k, n] = lnsc[n, i].  No DRAM round
trip, no transpose.  The broadcast matmuls + PSUM->SBUF copy for stage
k are emitted early in stage k+1's chunk loop so the PE queue never
head-of-line blocks on them.

Schedule: phase N for batch 0 runs first; batch 1's normalize chains
are injected into stage (0,0)'s chunk loop (they only gate stage 3).
F(k-1) pieces (T-add, max8 x8, tail) are interleaved into the back
half of stage k's chunk loop, so only F(last) drains at the end.
Part of each T-add / q2 slab-add runs on the otherwise-idle GpSimd
engine (SBUF-only operands).
"""
import numpy as np

import concourse.bass as bass
import concourse.mybir as mybir
from concourse import bacc
from concourse.tile import TileContext
from concourse.bass_utils import run_bass_kernel_spmd

F32 = mybir.dt.float32
FP16 = mybir.dt.float16
AF = mybir.ActivationFunctionType
ALU = mybir.AluOpType

B_FULL = 16
N_CORES = 8
B_LOC = B_FULL // N_CORES  # 2
C = 128
H = W = 64
NPH = 32
NP = 1024
PS = 2
TOPK = 3
NCHUNK = NP // 128  # 8
BAND = 512

N_GP_TADD = 1   # chunks (of 4) per T-add half that run on GpSimd
N_GP_S2 = 1     # chunks (of 4) per q2 slab-add group that run on GpSimd

LAST_EXEC_NS = None


def _band_c0(i: int) -> int:
    return min(max(128 * i - 192, 0), NP - BAND)


def _free_bcast_ap(tile_ap, free_dims):
    ap = tile_ap
    new = [ap.ap[0]] + [list(d) for d in free_dims]
    return bass.AP(ap.tensor, ap.offset, new)


def build_nc():
    import concourse.bacc as _bacc_mod
    _orig_tables = _bacc_mod.get_activation_tables

    def _one_table(arch):
        t = _orig_tables(arch)
        # keep dict order (act_func_set_id = index) but leave only the
        # ln+exp superset populated so every activation shares one table
        return {k: (v if k == "natural_log_exp_and_others" else set())
                for k, v in t.items()}

    _bacc_mod.get_activation_tables = _one_table
    try:
        return _build_nc_inner()
    finally:
        _bacc_mod.get_activation_tables = _orig_tables


def _build_nc_inner():
    nc = bacc.Bacc(trn_type="TRN2")

    x_d = nc.dram_tensor("x", [B_LOC, C, H * W], F32, kind="ExternalInput")
    alpha_d = nc.dram_tensor("alpha", [128, 1], F32, kind="ExternalInput")
    mask_d = nc.dram_tensor("mask", [128, NCHUNK, BAND], FP16,
                            kind="ExternalInput")
    ident_d = nc.dram_tensor("ident", [128, 128], FP16, kind="ExternalInput")
    out_d = nc.dram_tensor("out", [B_LOC, 3, NP, TOPK], F32, kind="ExternalOutput")

    with TileContext(nc) as tc:
        with tc.tile_pool(name="const", bufs=1) as cpool, \
             tc.tile_pool(name="zp", bufs=1) as zpool, \
             tc.tile_pool(name="slab", bufs=2) as slabp, \
             tc.tile_pool(name="ssl", bufs=4) as sslp, \
             tc.tile_pool(name="work", bufs=3) as work, \
             tc.tile_pool(name="wsc", bufs=3) as wscp, \
             tc.tile_pool(name="small", bufs=3) as small, \
             tc.tile_pool(name="ps", bufs=2, space="PSUM") as ps, \
             tc.tile_pool(name="psn", bufs=2, space="PSUM") as psn:

            # ---- input DMAs (xs0 first: it gates the critical path)
            xs_t = {}
            for b in range(B_LOC):
                xs_t[b] = slabp.tile([128, H * W], F32, name=f"xs{b}",
                                     tag="xs")
            for h in range(2):  # chunked halves
                nc.sync.dma_start(xs_t[0][:, 2048 * h:2048 * (h + 1)],
                                  x_d[0][:, 2048 * h:2048 * (h + 1)])

            ones_k = cpool.tile([128, 1], FP16)   # colsum matmul lhsT
            nc.vector.memset(ones_k, 1.0)
            ones_r = cpool.tile([1, 128], FP16)   # K=1 bcast matmul lhsT
            nc.vector.memset(ones_r, 1.0)
            av = cpool.tile([128, 1], F32)        # alpha
            nc.sync.dma_start(av, alpha_d[:, :])
            ident = cpool.tile([128, 128], FP16)
            nc.sync.dma_start(ident, ident_d[:, :])
            av_h = cpool.tile([128, 1], F32)      # alpha/2
            nc.vector.tensor_scalar_mul(av_h, av, 0.5)
            av_d = cpool.tile([128, 1], F32)      # 2*alpha
            nc.vector.tensor_scalar_mul(av_d, av, 2.0)
            rav2n = cpool.tile([128, 1], F32)     # -1/(2*alpha)
            nc.vector.reciprocal(rav2n, av_d)
            nc.vector.tensor_scalar_mul(rav2n, rav2n, -1.0)
            rav1n = cpool.tile([128, 1], F32)     # -1/alpha
            nc.vector.reciprocal(rav1n, av)
            nc.vector.tensor_scalar_mul(rav1n, rav1n, -1.0)
            scale_E = [av, av, av_h]        # a_q for the rowsum exp
            scale_T = [av_d, av_d, av]      # 2 a_q for the tiny value exp
            scale_Ln = [rav2n, rav2n, rav1n]  # -1/(2 a_q): ln(u) from ln(R)

            # ---- mask band (fp16, [p, i, j] with j a 512 window per chunk)
            mask_sb = cpool.tile([128, NCHUNK, BAND], FP16)
            nc.sync.dma_start(mask_sb, mask_d[:, :, :])

            for h in range(2):
                nc.sync.dma_start(xs_t[1][:, 2048 * h:2048 * (h + 1)],
                                  x_d[1][:, 2048 * h:2048 * (h + 1)])

            # ---- phase N pieces (emitted per batch; b=1 injected later)
            nrms = {}
            lnns = {}
            inv = {}
            zp = {}

            def xview(b):
                return xs_t[b].rearrange("c (i r j s) -> c r s i j",
                                         r=PS, s=PS, j=NPH)

            def n_zsq(b, p):
                zv = xview(b)[:, p, p]
                zsq = work.tile([128, NP], FP16, name="zsq", tag="zsq",
                                bufs=2)
                zsr = zsq.rearrange("c (a b) -> c a b", a=NPH)
                for h in range(2):  # halves pipeline with the x DMA chunks
                    nc.vector.tensor_tensor(
                        out=zsr[:, 16 * h:16 * (h + 1)],
                        in0=zv[:, 16 * h:16 * (h + 1)],
                        in1=zv[:, 16 * h:16 * (h + 1)], op=ALU.mult)
                nrm = psn.tile([1, NP], F32, name="nrm", tag="nrm", bufs=1)
                for h in range(2):
                    nc.tensor.matmul(nrm[:, 512 * h:512 * (h + 1)], ones_k,
                                     zsq[:, 512 * h:512 * (h + 1)],
                                     start=True, stop=True)
                nrms[(b, p)] = nrm

            def n_ln(b, p):
                lnn = small.tile([1, NP], F32, name="lnn", tag="lnn", bufs=2)
                nc.scalar.activation(lnn, nrms[(b, p)], AF.Ln)
                lnns[(b, p)] = lnn

            def n_inv(b, p):  # inv = exp(-0.5 ln nrm2)
                inv1 = small.tile([1, NP], FP16, name="inv1", tag="inv1",
                                  bufs=2)
                nc.scalar.activation(inv1, lnns[(b, p)], AF.Exp, scale=-0.5)
                inv[(b, p)] = inv1

            def n_z(b, p):
                ibc = psn.tile([128, NP], F32, name="ibc", tag="ibc", bufs=1)
                for h in range(2):
                    nc.tensor.matmul(ibc[:, 512 * h:512 * (h + 1)], ones_r,
                                     inv[(b, p)][:, 512 * h:512 * (h + 1)],
                                     start=True, stop=True)
                z = zpool.tile([128, NP], FP16, name=f"z{b}{p}",
                               tag=f"z{b}{p}", bufs=1)
                nc.vector.tensor_tensor(
                    out=z.rearrange("c (a b) -> c a b", a=NPH),
                    in0=xview(b)[:, p, p],
                    in1=ibc.rearrange("c (a b) -> c a b", a=NPH), op=ALU.mult)
                zp[(b, p)] = z

            # batch 0 chains now (critical path to first stage)
            for p in range(PS):
                n_zsq(0, p)
            for p in range(PS):
                n_ln(0, p)
                n_inv(0, p)
            for p in range(PS):
                n_z(0, p)

            # batch 1 chain pieces, injected into stage (0,0)'s chunk loop
            b1_pieces = [
                lambda: n_zsq(1, 0),
                lambda: n_zsq(1, 1),
                lambda: (n_ln(1, 0), n_inv(1, 0), n_z(1, 0)),
                lambda: (n_ln(1, 1), n_inv(1, 1), n_z(1, 1)),
            ]

            # ---- phase M: per-chunk interleaved software pipeline
            s_of = {}  # (b, q) -> slab

            def E_chunk(b, q, i, s_sl, rT):
                if q < 2:
                    zs = zp[(b, q)]
                    G = ps.tile([128, NP], F32, name="G", tag="G")
                    for h in range(2):
                        nc.tensor.matmul(
                            G[:, 512 * h:512 * (h + 1)],
                            zs[:, 128 * i:128 * (i + 1)],
                            zs[:, 512 * h:512 * (h + 1)],
                            start=True, stop=True)
                    # mask==1 outside a 512-wide band around the diagonal
                    # blocks: multiply only the band (DVE), copy the rest
                    c0 = _band_c0(i)
                    nc.vector.tensor_tensor(
                        out=s_sl[:, i, c0:c0 + BAND],
                        in0=G[:, c0:c0 + BAND],
                        in1=mask_sb[:, i, :], op=ALU.mult)
                    for a, bnd in ((0, c0), (c0 + BAND, NP)):
                        if bnd > a:
                            nc.scalar.copy(s_sl[:, i, a:bnd], G[:, a:bnd])
                else:
                    s0, s1 = s_of[(b, 0)], s_of[(b, 1)]
                    if i % 4 == 0:  # batched 4-chunk add, split DVE/GpSimd
                        nd = 4 - N_GP_S2
                        nc.vector.tensor_tensor(
                            out=s_sl[:, i:i + nd, :], in0=s0[:, i:i + nd, :],
                            in1=s1[:, i:i + nd, :], op=ALU.add)
                        if N_GP_S2:
                            nc.gpsimd.tensor_tensor(
                                out=s_sl[:, i + nd:i + 4, :],
                                in0=s0[:, i + nd:i + 4, :],
                                in1=s1[:, i + nd:i + 4, :], op=ALU.add)
                e_scr = work.tile([128, NP], FP16, name="e_scr", tag="e_scr",
                                  bufs=2)
                nc.scalar.activation(e_scr, s_sl[:, i, :], AF.Exp,
                                     scale=scale_E[q],
                                     accum_out=rT[:, i:i + 1])

            def E_tail_sums(b, q, s_sl, rT):
                u8 = small.tile([128, NCHUNK], F32, name="u8", tag="u8",
                                bufs=3)
                nc.vector.reciprocal(u8, rT)
                lnr = small.tile([128, NCHUNK], F32, name="lnr", tag="lnr")
                nc.scalar.activation(lnr, rT, AF.Ln)
                lnsc = small.tile([128, NCHUNK], FP16, name="lnsc",
                                  tag="lnsc", bufs=2)
                nc.vector.tensor_scalar_mul(lnsc, lnr, scale_Ln[q][:, 0:1])
                return dict(s_sl=s_sl, lnsc=lnsc, u8=u8, b=b, q=q)

            def E_tail_bcast(stg):
                # lnubc[p, 128*i + n] = lnsc[n, i] via 8 identity matmuls
                lnps = psn.tile([128, NP], F32, name="lnps", tag="ibc",
                                bufs=1)
                for i in range(NCHUNK):
                    lhsT = _free_bcast_ap(stg["lnsc"][:, i:i + 1], [[0, 128]])
                    nc.tensor.matmul(lnps[:, 128 * i:128 * (i + 1)],
                                     lhsT, ident, start=True, stop=True)
                lnubc = wscp.tile([128, NP], FP16, name="lnubc", tag="lnubc",
                                  bufs=2)
                nc.scalar.copy(lnubc, lnps)
                stg["lnubc"] = lnubc

            def F_thalf(stg, h, t8):
                # batched T-add over 4 chunks: T = s + ln(u_m)/(2 a_q)
                T_bat = wscp.tile([128, 4, NP], FP16, name=f"T_bat{h}",
                                  tag=f"T_bat{h}", bufs=2)
                nd = 4 - N_GP_TADD
                lnb = _free_bcast_ap(stg["lnubc"][:, :], [[0, nd], [1, NP]])
                nc.vector.tensor_tensor(
                    out=T_bat[:, :nd, :],
                    in0=stg["s_sl"][:, 4 * h:4 * h + nd, :],
                    in1=lnb, op=ALU.add)
                if N_GP_TADD:
                    lnb2 = _free_bcast_ap(stg["lnubc"][:, :],
                                          [[0, N_GP_TADD], [1, NP]])
                    nc.gpsimd.tensor_tensor(
                        out=T_bat[:, nd:, :],
                        in0=stg["s_sl"][:, 4 * h + nd:4 * (h + 1), :],
                        in1=lnb2, op=ALU.add)
                return T_bat

            def F_max8(stg, i, T_bat, t8):
                nc.vector.max(out=t8[:, i, :], in_=T_bat[:, i % 4, :])

            def F_tail(stg, t8):
                b, q = stg["b"], stg["q"]
                tex = small.tile([128, NCHUNK, TOPK], F32, name="tex",
                                 tag="tex")
                nc.scalar.activation(tex, t8[:, :, :TOPK], AF.Exp,
                                     scale=scale_T[q])
                oacc = small.tile([128, NCHUNK, TOPK], F32, name="oacc",
                                  tag="oacc")
                u8b = _free_bcast_ap(stg["u8"][:, :], [[1, NCHUNK], [0, TOPK]])
                nc.vector.tensor_tensor(out=oacc, in0=tex, in1=u8b,
                                        op=ALU.mult)
                dst = out_d[b, q].rearrange("(i p) k -> p i k", p=128)
                nc.sync.dma_start(dst, oacc)

            def F_pieces(stg, t8):
                """Generator of F-phase emission pieces for one stage."""
                holder = {}

                def thalf(h):
                    def run():
                        holder[h] = F_thalf(stg, h, t8)
                    return run

                def m8(j):
                    def run():
                        F_max8(stg, j, holder[j // 4], t8)
                    return run

                yield thalf(0)
                for j in range(4):
                    yield m8(j)
                yield thalf(1)
                for j in range(4, 8):
                    yield m8(j)
                yield (lambda: F_tail(stg, t8))

            # per-chunk slots for F(k-1) pieces within stage k's loop
            F_SLOT = {3: 2, 4: 2, 5: 2, 6: 2, 7: 3}

            stages = [(b, q) for b in range(B_LOC) for q in range(3)]
            pend = []     # stages awaiting lnubc broadcast emission
            prevF = None  # piece iterator of F(k-1)
            for k, (b, q) in enumerate(stages):
                s_sl = sslp.tile([128, NCHUNK, NP], FP16, name="s_sl",
                                 tag="s_sl")
                s_of[(b, q)] = s_sl
                rT = small.tile([128, NCHUNK], F32, name="rT", tag="rT")
                for i in range(NCHUNK):
                    E_chunk(b, q, i, s_sl, rT)
                    if i == 1 and pend:
                        E_tail_bcast(pend.pop(0))
                    if k == 0 and i in (4, 5, 6, 7):
                        b1_pieces[i - 4]()
                    if prevF is not None:
                        for _ in range(F_SLOT.get(i, 0)):
                            piece = next(prevF, None)
                            if piece is None:
                                break
                            piece()
                if prevF is not None:  # drain leftovers (shouldn't happen)
                    for piece in prevF:
                        piece()
                stg = E_tail_sums(b, q, s_sl, rT)
                pend.append(stg)
                t8 = small.tile([128, NCHUNK, 8], FP16, name="t8", tag="t8",
                                bufs=3)
                prevF = F_pieces(stg, t8)
            for stg in pend:
                E_tail_bcast(stg)
            for piece in prevF:
                piece()

    nc.compile()
    return nc


_NC_CACHE = None


def _get_nc():
    global _NC_CACHE
    if _NC_CACHE is None:
        _NC_CACHE = build_nc()
    return _NC_CACHE


def _build_mask_band() -> np.ndarray:
    rat_s = np.float32(0.05)
    sr = np.float32(NPH) * rat_s
    ind_r = np.arange(NPH, dtype=np.float32).reshape(1, NPH, 1)
    ind_c = np.arange(NPH, dtype=np.float32).reshape(1, 1, NPH)
    cent = np.arange(NPH, dtype=np.float32)
    cent_r = np.repeat(cent, NPH).reshape(NP, 1, 1)
    cent_c = np.tile(cent, NPH).reshape(NP, 1, 1)
    g = np.exp(-((ind_r - cent_r) ** 2) / (2.0 * sr * sr)) * np.exp(
        -((ind_c - cent_c) ** 2) / (2.0 * sr * sr)
    )
    full = (1.0 - g).reshape(NP, NP).astype(np.float16)
    band = np.empty((128, NCHUNK, BAND), dtype=np.float16)
    for i in range(NCHUNK):
        c0 = _band_c0(i)
        band[:, i, :] = full[128 * i:128 * (i + 1), c0:c0 + BAND]
    return band


def kernel(x: np.ndarray, alpha: np.ndarray) -> np.ndarray:
    global LAST_EXEC_NS
    x = np.ascontiguousarray(np.asarray(x, dtype=np.float32))
    alpha_arr = np.full((128, 1), np.float32(np.asarray(alpha)),
                        dtype=np.float32)
    mask = _build_mask_band()
    ident = np.eye(128, dtype=np.float16)

    nc = _get_nc()
    in_maps = []
    for core in range(N_CORES):
        xs = x[core * B_LOC:(core + 1) * B_LOC].reshape(B_LOC, C, H * W)
        in_maps.append({"x": np.ascontiguousarray(xs), "alpha": alpha_arr,
                        "mask": mask, "ident": ident})
    res = run_bass_kernel_spmd(nc, in_maps, core_ids=list(range(N_CORES)))
    LAST_EXEC_NS = res.exec_time_ns

    out = np.empty((B_FULL, TOPK, H, W), dtype=np.float32)
    for core in range(N_CORES):
        t = res.results[core]["out"]
        for bl in range(B_LOC):
            bg = core * B_LOC + bl
            tq = t[bl].reshape(3, NPH, NPH, TOPK).transpose(0, 3, 1, 2)
            out[bg, :, 0::2, 0::2] = tq[0]
            out[bg, :, 1::2, 1::2] = tq[1]
            out[bg, :, 0::2, 1::2] = tq[2]
            out[bg, :, 1::2, 0::2] = tq[2]
    return out
